# revision 68
# baseline (speedup 1.0000x reference)
"""Trainium2 Bass kernel for the diagonal OU-SDE sampler (nn_DiagOUSDE).

Math: y[b,0]=mu+noise[b,0]*sqrt(var0); y[b,t]=A_t*y[b,t-1]+mu(1-A_t)+sqrt(Q_t)*noise[b,t]
with A/Q per (t,d) exact OU transition coefficients.

Fast path ("fastb", used when the coefficients are d-uniform): batch-sharded
8 ways; per core 8 rows processed as 4 pairs (matmul N=512). Time is chunked
into C=22 chunks of L=96; each chunk's scan is one PE matmul against a
[96,97] bf16 folded weight (W[s,t] = sqrtQ_{t0+s} * prod A over (s,t]), with
output columns rolled so PSUM row 0 = the chunk-final y (the carry) and rows
1..96 = y. The cross-chunk carry is a rank-1 PE matmul (weight = per-chunk
decay profile [1,97], rhs = previous chunk's evacuated SBUF row 0 — a legal
PE rhs base partition) accumulated into the same PSUM bank, so the only
per-chunk PSUM->SBUF evacuation is a single [97,512] fp32->bf16 copy that
alternates between DVE and ScalarE.

Memory-bound: all DRAM traffic is bf16 (noise in, y out: 8.7+8.7 MB/core at
rel-err ~4e-3), in fully-contiguous 1.08MB half-pair DMAs. SP issues every
load and Pool (SWDGE) every store so neither blocks the other's queue, tiles
are double-buffered per (pair, half), and the K-iteration timing loop unrolls
_UNROLL bodies per For_i iteration to amortize the loop's all-engine
semaphore-reset barrier; bodies pipeline across the unrolled group.
"""
import sys

for _p in ("/opt/trn_rl_repo", "/opt/pypackages"):
    if _p not in sys.path:
        sys.path.append(_p)

import numpy as np

import concourse.bacc as bacc
import concourse.mybir as mybir
from concourse.tile import TileContext
from concourse.bass_utils import run_bass_kernel_spmd

B, T, D = 64, 2048, 256
N_CORES = 8
B_S = B // N_CORES            # 8 batch rows per core
L = 96                        # time steps per chunk; contraction row 96 = carry
C = (T + L - 1) // L          # 22 chunks
TP = C * L                    # padded time length 2112

_f32 = np.float32


def _host_coeffs(ts, mu, log_kappa, log_sigma):
    """Per-(t,d) coefficient arrays in float32, mirroring the JAX reference."""
    ts = ts.astype(_f32)
    kappa = np.logaddexp(_f32(0.0), log_kappa.astype(_f32)).astype(_f32) + _f32(1e-6)
    sigma = np.logaddexp(_f32(0.0), log_sigma.astype(_f32)).astype(_f32) + _f32(1e-6)
    var0 = sigma * sigma / (_f32(2.0) * kappa)
    dt = np.maximum(ts[1:] - ts[:-1], _f32(1e-6))[:, None]            # [T-1,1]
    A = np.exp(-kappa[None, :] * dt).astype(_f32)                     # [T-1,D]
    two_k_dt = (_f32(2.0) * kappa[None, :] * dt).astype(_f32)
    small = (two_k_dt < _f32(1e-6)).astype(_f32)
    Q_exact = sigma**2 * (_f32(1.0) - np.exp(-two_k_dt)) / np.maximum(
        _f32(2.0) * kappa, _f32(1e-12))
    Q_taylor = sigma**2 * dt * (_f32(1.0) - kappa * dt + two_k_dt**2 / _f32(6.0))
    Q = (small * Q_taylor + (_f32(1.0) - small) * Q_exact).astype(_f32)

    A_full = np.concatenate([np.ones((1, D), _f32), A], axis=0)       # A_0 := 1
    sqrtQ_full = np.sqrt(
        np.concatenate([var0[None, :], Q], axis=0)).astype(_f32)      # [T,D]
    b_full = np.concatenate(
        [mu[None, :].astype(_f32), (mu[None, :] * (_f32(1.0) - A)).astype(_f32)],
        axis=0)

    logG = np.cumsum(np.log(A_full.astype(np.float64)), axis=0)
    G = np.exp(logG).astype(_f32)
    S_u = (sqrtQ_full * np.exp(-logG)).astype(_f32)                   # u = noise*S_u

    if np.any(b_full != 0):
        ydet = np.empty((T, D), _f32)
        y = b_full[0].copy()
        ydet[0] = y
        for t in range(1, T):
            y = A_full[t] * y + b_full[t]
            ydet[t] = y
    else:
        ydet = None
    return S_u, G, ydet, A_full, sqrtQ_full


def _pad_tp(a):
    out = np.zeros((TP, D), _f32)
    out[:T] = a
    return out


def _tri_weight():
    # [97, 97]: W[s,t]=1{s<=t} (s,t<96); row 96 = carry (all ones);
    # col 96 duplicates col 95 => psum row 96 = chunk-final cum (the next carry)
    w = np.zeros((97, 97), _f32)
    for s in range(L):
        w[s, s:L] = _f32(1.0)
    w[L, :L] = _f32(1.0)
    w[:, L] = w[:, L - 1]
    return w


def _wfold_weights(A_full, sqrtQ_full):
    """[C, 97, 97] float32 exact per-chunk transition weights (d-independent
    coefficients; requires per-t d-uniform A/sqrtQ). The sqrt(Q_s) input scaling
    is folded into the weight rows, so the rhs is RAW noise and PSUM rows are
    final y: W_c[s,t] = (prod_{r=t0+s+1..t0+t} A_r) * sqrtQ_{t0+s} for s<=t<96,
    row 96 = carry coefficients prod_{t0..t0+t} A_r, col 96 duplicates col 95
    (so PSUM row 96 = chunk-final y = the next chunk's carry)."""
    q = np.zeros(TP)
    q[:T] = sqrtQ_full[:, 0]
    ap = np.zeros(TP)
    ap[:T] = A_full[:, 0].astype(np.float64)
    Ws = np.zeros((C, L + 1, L + 1), np.float64)
    for c in range(C):
        t0 = c * L
        with np.errstate(divide="ignore"):
            cls = np.cumsum(np.log(ap[t0:t0 + L]))  # log prod_{t0..t0+t}
        M = np.exp(cls[:, None] - cls[None, :])     # [t, s] = prod_{s+1..t}
        M = np.tril(M)
        np.fill_diagonal(M, 1.0)
        W = Ws[c]
        W[:L, :L] = (M * q[t0:t0 + L][None, :]).T   # W[s,t] = M[t,s]*q[t0+s]
        W[L, :L] = np.exp(cls)
        W[:, L] = W[:, L - 1]
    return np.ascontiguousarray(Ws.astype(_f32))


_BUILD_VARIANT = "full"  # bench hook: "full" | "dma_only" | "compute_only"
_UNROLL = 8              # bodies per For_i iteration (barrier amortization)
_FAST_MODE = "fastc"     # "fastb" (L=96, 22 links) | "fastc" (L=126, 17 links)
_DMA_GRAN = "half"       # "half" (8+8 x 1.08MB per iter; best on HW) | "pair"

# fastc geometry: long chunks -> 17 chunks, ~23% fewer matmuls/evacs than
# L=96. L2=126 (127 weight cols) deliberately avoids NumWeights==128, which
# would enable Fast Weight Load: L2=127/128-col weights measured 3.4x SLOWER
# on real HW than the cost model predicts.
L2 = 126
C2 = 17                       # 16 full chunks + one 32-step tail chunk
TP2 = C2 * L2                 # 2142
CA, CB = 9, 8                 # chunk split for half-granularity loads/stores


def _wfold_weights2(A_full, sqrtQ_full):
    """[C2, 128, 128] folded chunk weights for L=127 chunks, with output
    columns rolled by 1 (col 0 = dup'd chunk-final y = the carry). Rows
    0..126 contract the noise; row 127 is the carry decay profile."""
    q = np.zeros(TP2)
    q[:T] = sqrtQ_full[:, 0]
    ap = np.zeros(TP2)
    ap[:T] = A_full[:, 0].astype(np.float64)
    Ws = np.zeros((C2, L2 + 1, L2 + 1), np.float64)
    for c in range(C2):
        t0 = c * L2
        with np.errstate(divide="ignore", invalid="ignore"):
            cls = np.cumsum(np.log(ap[t0:t0 + L2]))
            M = np.exp(cls[:, None] - cls[None, :])  # [t, s] = prod_{s+1..t}
        M = np.tril(M)
        np.fill_diagonal(M, 1.0)
        W = Ws[c]
        W[:L2, :L2] = (M * q[t0:t0 + L2][None, :]).T  # W[s,t]=M[t,s]*q[t0+s]
        W[L2, :L2] = np.exp(cls)
        W[:, L2] = W[:, L2 - 1]
    Ws = np.roll(Ws, 1, axis=2)   # col 0 = carry, cols 1..127 = y[t0..]
    return np.ascontiguousarray(Ws.astype(_f32))


def _build_nc_fastc(n_iters=1):
    """L=127 chunk variant of the bf16 fast path (see _build_nc_fastb)."""
    nc = bacc.Bacc("TRN2", target_bir_lowering=False, debug=False,
                   num_devices=N_CORES)
    dt32 = mybir.dt.float32
    dtb = mybir.dt.bfloat16
    NP = B_S // 2
    noiseA = nc.dram_tensor("noiseA", [NP, L2, CA, 2, D], dtb,
                            kind="ExternalInput")
    noiseB = nc.dram_tensor("noiseB", [NP, L2, CB, 2, D], dtb,
                            kind="ExternalInput")
    w1stack = nc.dram_tensor("w1stack", [L2, C2 * (L2 + 1)], dtb,
                             kind="ExternalInput")
    w2stack = nc.dram_tensor("w2stack", [1, C2 * (L2 + 1)], dtb,
                             kind="ExternalInput")
    wbstack = nc.dram_tensor("wbstack", [L2, CA], dtb, kind="ExternalInput")
    youtA = nc.dram_tensor("youtA", [NP, L2, CA, 2, D], dtb,
                           kind="ExternalOutput")
    youtB = nc.dram_tensor("youtB", [NP, L2, CB, 2, D], dtb,
                           kind="ExternalOutput")

    with TileContext(nc) as tc:
        with (
            tc.tile_pool(name="coef", bufs=1) as coef,
            tc.tile_pool(name="upoolA", bufs=2 * NP) as upoolA,
            tc.tile_pool(name="upoolB", bufs=2 * NP) as upoolB,
            tc.tile_pool(name="bcpool", bufs=2) as bcpool,
            tc.tile_pool(name="psum", bufs=8, space="PSUM") as pspool,
        ):
            w1_t = coef.tile([L2, C2, L2 + 1], dtb, tag="w1", name="w1_t")
            w2_t = coef.tile([1, C2, L2 + 1], dtb, tag="w2", name="w2_t")
            wb_t = coef.tile([L2, CA], dtb, tag="wb", name="wb_t")
            nc.scalar.dma_start(
                out=w1_t[:],
                in_=w1stack[:].rearrange("s (c t) -> s c t", c=C2))
            nc.scalar.dma_start(
                out=w2_t[:],
                in_=w2stack[:].rearrange("s (c t) -> s c t", c=C2))
            nc.scalar.dma_start(out=wb_t[:], in_=wbstack[:])
            wps = pspool.tile([L2 + 1, 2 * D], dt32, tag="ps", name="wps")
            for _ in range(10):
                nc.tensor.matmul(wps[:, 0:L2 + 1], w1_t[:, 0, :],
                                 w1_t[:, 0, :], start=True, stop=True)

            def body(_iv=None):
                us = [[None, None] for _ in range(NP)]
                for half in range(2):
                    for p in range(NP):
                        pool = upoolA if half == 0 else upoolB
                        cw = CA if half == 0 else CB
                        u = pool.tile([128, cw, 2, D], dtb, tag="u",
                                      name=f"u{p}h{half}")
                        us[p][half] = u
                        src = noiseA if half == 0 else noiseB
                        eng = nc.scalar if p >= 2 else nc.sync
                        eng.dma_start(out=u[0:L2, :, :, :],
                                      in_=src[p, :, :, :, :])

                # Split chains: per pair, the front (chunks 0..CA-1) and
                # back (CA..C2-1) chains run CONCURRENTLY. The back chain's
                # seed carry is computed by CA accumulating [126->1]
                # matmuls over the front tile's RAW noise, emitted at the
                # FRONT chain's first step — i.e. in program order BEFORE
                # any front evac can overwrite that noise (the dead-slice
                # aliasing makes later noise reads invalid).
                bcar = bcpool.tile([1, NP, 2, D], dtb, tag="bc", name="bcar")
                chains = []
                for p in range(NP):
                    chains.append((p, 0, CA, None))       # front
                    chains.append((p, CA, C2, "b"))       # back, seeded
                SKEW = 2
                for step in range(CB + (len(chains) - 1) * SKEW):
                    for k, (p, c0, c1, seed) in enumerate(chains):
                        c = c0 + step - k * SKEW
                        if not (c0 <= c < c1):
                            continue
                        ut = us[p][0 if c < CA else 1]
                        cc = c if c < CA else c - CA
                        if c == 0 and seed is None:
                            # boundary-seed accumulation from pristine noise
                            bps = pspool.tile([L2 + 1, 2 * D], dt32,
                                              tag="ps", name=f"bps{p}")
                            for bc in range(CA):
                                nc.tensor.matmul(
                                    bps[0:1, :], wb_t[:, bc:bc + 1],
                                    ut[0:L2, bc, :, :],
                                    start=(bc == 0), stop=(bc == CA - 1),
                                    skip_group_check=True)
                            nc.vector.tensor_copy(
                                out=bcar[0:1, p, :, :], in_=bps[0:1, :])
                        ps = pspool.tile([L2 + 1, 2 * D], dt32,
                                         tag="ps", name=f"ps{p}{c % 2}")
                        nc.tensor.matmul(ps[:], w1_t[:, c, :],
                                         ut[0:L2, cc, :, :],
                                         start=True, stop=(c == 0),
                                         skip_group_check=True)
                        if c == c0 and seed == "b":
                            nc.tensor.matmul(
                                ps[:], w2_t[:, c, :], bcar[0:1, p, :, :],
                                start=False, stop=True, skip_group_check=True)
                        elif c > c0:
                            cp = c - 1
                            up = us[p][0 if cp < CA else 1]
                            nc.tensor.matmul(
                                ps[:], w2_t[:, c, :],
                                up[0:1, cp if cp < CA else cp - CA, :, :],
                                start=False, stop=True, skip_group_check=True)
                        if (c % 2) == 0:
                            nc.vector.tensor_copy(out=ut[0:L2 + 1, cc, :, :],
                                                  in_=ps[:])
                        else:
                            nc.scalar.copy(ut[0:L2 + 1, cc, :, :], ps[:])
                        if c == CA - 1 or c == C2 - 1:
                            half = 0 if c == CA - 1 else 1
                            dst = youtA if half == 0 else youtB
                            nc.gpsimd.dma_start(out=dst[p, :, :, :, :],
                                                in_=ut[1:L2 + 1, :, :, :])

            if n_iters == 1:
                body()
            else:
                U = _UNROLL if (n_iters - 1) % _UNROLL == 0 else 1
                body()
                with tc.For_i(0, (n_iters - 1) // U, 1) as _i:
                    for _r in range(U):
                        body()
    nc.compile()
    return nc


def _build_nc_fastb(n_iters=1):
    """bf16-I/O fast path. All DRAM traffic is bf16 (noise in, folded weights,
    y out); PSUM stays fp32. The carry is no longer injected as contraction
    row 96 of the rhs — instead a rank-1 matmul (weight = per-chunk decay
    profile [1,97], rhs = previous chunk's evacuated SBUF row 96) accumulates
    it into PSUM. That folds the old per-chunk [1,512] carry copy into the
    single [97,512] PSUM->SBUF evac (DVE/ScalarE alternating, fp32->bf16
    converting), whose row 96 is the next chunk's carry.
    """
    nc = bacc.Bacc("TRN2", target_bir_lowering=False, debug=False,
                   num_devices=N_CORES)
    dt32 = mybir.dt.float32
    dtb = mybir.dt.bfloat16
    NP = B_S // 2  # batch pairs
    CH = C // 2    # chunk half-point (loads/stores split for finer overlap)
    pair_gran = _DMA_GRAN == "pair"
    # noise host-permuted so each DMA is one fully contiguous transfer on
    # BOTH the DRAM and SBUF side: "pair" = one 2.16MB DMA per batch pair
    # (fewest per-DMA fixed costs), "half" = 1.08MB chunk-halves
    if pair_gran:
        noisep = nc.dram_tensor("noisep", [NP, L, C, 2, D], dtb,
                                kind="ExternalInput")
        youtp = nc.dram_tensor("youtp", [NP, L, C, 2, D], dtb,
                               kind="ExternalOutput")
    else:
        noisep = nc.dram_tensor("noisep", [NP, 2, L, CH, 2, D], dtb,
                                kind="ExternalInput")
        youtp = nc.dram_tensor("youtp", [NP, 2, L, CH, 2, D], dtb,
                               kind="ExternalOutput")
    # w1: noise rows of the folded chunk weights, [s, c, t]; w2: carry decay rows
    w1stack = nc.dram_tensor("w1stack", [L, C * (L + 1)], dtb,
                             kind="ExternalInput")
    w2stack = nc.dram_tensor("w2stack", [1, C * (L + 1)], dtb,
                             kind="ExternalInput")

    with TileContext(nc) as tc:
        with (
            tc.tile_pool(name="coef", bufs=1) as coef,
            # 2-body double buffering either way (~180KB/partition)
            tc.tile_pool(name="upool",
                         bufs=2 * NP if pair_gran else 4 * NP) as upool,
            tc.tile_pool(name="psum", bufs=8, space="PSUM") as pspool,
        ):
            # constant weights: load ONCE, outside the timing loop, kicked
            # from ACT (idle early) so SP's noise-load chain starts at t=0
            w1_t = coef.tile([L, C, L + 1], dtb, tag="w1", name="w1_t")
            w2_t = coef.tile([1, C, L + 1], dtb, tag="w2", name="w2_t")
            nc.scalar.dma_start(
                out=w1_t[:], in_=w1stack[:].rearrange("s (c t) -> s c t", c=C))
            nc.scalar.dma_start(
                out=w2_t[:], in_=w2stack[:].rearrange("s (c t) -> s c t", c=C))
            # PE warmup, once, pre-loop: ramps the clock gate so the first
            # chain matmuls run at full rate
            wps = pspool.tile([L + 1, 2 * D], dt32, tag="ps", name="wps")
            for _ in range(10):
                nc.tensor.matmul(wps[:, 0:L + 1], w1_t[:, 0, :],
                                 w1_t[:, 0, :], start=True, stop=True)

            dma_on = _BUILD_VARIANT in ("full", "dma_only", "dma_split")
            compute_on = _BUILD_VARIANT in ("full", "compute_only")
            split_load = _BUILD_VARIANT in ("dma_split", "full")

            def body(_iv=None):
                # Tiles are double-buffered across bodies so loads of body
                # k+1 overlap stores of body k (pool FIFO recycling).
                # SP issues ALL loads and Pool ALL stores: an engine that
                # issued a late store would head-of-line block the next
                # For_i iteration's loads (in-order engine queues).
                us = [[None, None] for _ in range(NP)]
                if pair_gran:
                    for p in range(NP):
                        u = upool.tile([128, C, 2, D], dtb, tag="u",
                                       name=f"u{p}")
                        us[p][0] = us[p][1] = u
                        if dma_on:
                            nc.sync.dma_start(out=u[0:L, :, :, :],
                                              in_=noisep[p, :, :, :, :])
                else:
                    for half in range(2):
                        for p in range(NP):
                            u = upool.tile([128, CH, 2, D], dtb, tag="u",
                                           name=f"u{p}h{half}")
                            us[p][half] = u
                            if dma_on:
                                # HW DMAs serialize per issuing queue at
                                # ~250GB/s but overlap ACROSS queues: split
                                # loads SP/ACT (ACT takes the later-consumed
                                # pairs), stores stay on Pool
                                eng = (nc.scalar if split_load and p >= 2
                                       else nc.sync)
                                eng.dma_start(
                                    out=u[0:L, :, :, :],
                                    in_=noisep[p, half, :, :, :, :])
                # Skewed chain interleave: pair p runs SKEW chunks behind
                # pair p-1. MM1 is issued one chunk AHEAD of the carry chain
                # so only MM2+evac sit on the serial dependency.
                SKEW = 3
                pss = [dict() for _ in range(NP)]
                for step in range(C + (NP - 1) * SKEW):
                    for p in range(NP):
                        c = step - p * SKEW
                        if not (0 <= c < C):
                            continue
                        ut = us[p][c // CH]
                        cc = c if pair_gran else c % CH
                        if not compute_on:
                            if dma_on and pair_gran and c == C - 1:
                                nc.gpsimd.dma_start(
                                    out=youtp[p, :, :, :, :],
                                    in_=ut[1:L + 1, :, :, :])
                            elif dma_on and not pair_gran and (
                                    c == CH - 1 or c == C - 1):
                                half = 0 if c == CH - 1 else 1
                                nc.gpsimd.dma_start(
                                    out=youtp[p, half, :, :, :, :],
                                    in_=ut[1:L + 1, :, :, :])
                            continue
                        if c == 0:
                            ps = pspool.tile([L + 1, 2 * D], dt32,
                                             tag="ps", name=f"ps{p}a")
                            pss[p][0] = ps
                            # weight cols rolled by 1: PSUM row 0 = chunk-
                            # final y (next carry), rows 1..96 = y[t0..]
                            nc.tensor.matmul(ps[:], w1_t[:, 0, :],
                                             ut[0:L, 0, :, :],
                                             start=True, stop=True)
                        if c + 1 < C:
                            un = us[p][(c + 1) // CH]
                            ccn = c + 1 if pair_gran else (c + 1) % CH
                            nxt = pspool.tile([L + 1, 2 * D], dt32, tag="ps",
                                              name=f"ps{p}{'ab'[c % 2]}")
                            pss[p][c + 1] = nxt
                            nc.tensor.matmul(nxt[:], w1_t[:, c + 1, :],
                                             un[0:L, ccn, :, :],
                                             start=True, stop=False,
                                             skip_group_check=True)
                        if c > 0:
                            up = us[p][(c - 1) // CH]
                            ccp = c - 1 if pair_gran else (c - 1) % CH
                            ps = pss[p].pop(c)
                            # carry: rank-1 accumulate of prev chunk's final
                            # y (evac'd SBUF row 0, a legal PE rhs base
                            # partition) against the decay profile
                            nc.tensor.matmul(ps[:], w2_t[:, c, :],
                                             up[0:1, ccp, :, :],
                                             start=False, stop=True,
                                             skip_group_check=True)
                        # single evac: next-carry row 0 + y rows 1..96,
                        # fp32 PSUM -> bf16 SBUF into the dead u slice.
                        # ACT also issues 4 loads (~17us engine-blocked) at
                        # body start, so it gets only LATE chunks' evacs
                        # (early chain links stay all-DVE and never wait on
                        # the ACT load queue); DVE takes 14/22 per pair
                        if c >= 7 and c % 2 == 1:
                            nc.scalar.copy(ut[0:L + 1, cc, :, :], ps[:])
                        else:
                            nc.vector.tensor_copy(out=ut[0:L + 1, cc, :, :],
                                                  in_=ps[:])
                        if dma_on and pair_gran and c == C - 1:
                            nc.gpsimd.dma_start(out=youtp[p, :, :, :, :],
                                                in_=ut[1:L + 1, :, :, :])
                        elif dma_on and not pair_gran and (
                                c == CH - 1 or c == C - 1):
                            half = 0 if c == CH - 1 else 1
                            # half-stores interleave into the DMA stream
                            # during compute
                            nc.gpsimd.dma_start(
                                out=youtp[p, half, :, :, :, :],
                                in_=ut[1:L + 1, :, :, :])

            if n_iters == 1:
                body()
            else:
                # unroll: the For_i iteration boundary is a full all-engine
                # barrier (semaphore reset) that drains the pipeline; U
                # bodies per iteration amortize it while sems keep counting
                # (cross-body overlap) within the group
                U = _UNROLL if (n_iters - 1) % _UNROLL == 0 else 1
                body()
                with tc.For_i(0, (n_iters - 1) // U, 1) as _i:
                    for _r in range(U):
                        body()
    nc.compile()
    return nc


def _build_nc_fast(n_iters=1, f32r=False):
    """d-uniform fast path: exact per-chunk weights with the input scaling
    folded in — no elementwise pass over the noise at all. The PSUM->SBUF
    evacuation is a plain copy (split DVE/ScalarE) into the dead u slice.
    Batch rows are paired so matmuls run at N=512; all 4 pair-chains in flight.
    """
    nc = bacc.Bacc("TRN2", target_bir_lowering=False, debug=False,
                   num_devices=N_CORES)
    dt32 = mybir.dt.float32
    dtmm = mybir.dt.float32r if f32r else mybir.dt.float32
    # noise host-permuted per b to [L, C, D] (t = c*L + p -> [p, c, d]) so all
    # DRAM accesses are contiguous (strided DRAM APs measured 3.6x slower)
    noisep = nc.dram_tensor("noisep", [B_S, L, C, D], dtmm, kind="ExternalInput")
    # weight stack pre-transposed on host to [s, c, t] so the load is contiguous
    wstack = nc.dram_tensor("wstack", [L + 1, C * (L + 1)], dtmm,
                            kind="ExternalInput")
    youtp = nc.dram_tensor("youtp", [B_S, L, C, D], dt32, kind="ExternalOutput")
    NP = B_S // 2  # batch pairs
    CH = C // 2    # chunk half-point (loads/stores split for finer overlap)

    with TileContext(nc) as tc:
        with (
            tc.tile_pool(name="coef", bufs=1) as coef,
            tc.tile_pool(name="upool", bufs=NP) as upool,
            tc.tile_pool(name="psum", bufs=8, space="PSUM") as pspool,
        ):
            def body(_iv=None):
                w_t = coef.tile([L + 1, C, L + 1], dtmm, tag="w", name="w_t")
                nc.sync.dma_start(
                    out=w_t[:], in_=wstack[:].rearrange("s (c t) -> s c t", c=C))

                us = []
                halves = ((0, CH), (CH, C))
                for hi, (c0, c1) in enumerate(halves):
                    for p in range(NP):
                        if hi == 0:
                            u = upool.tile([128, C, 2, D], dtmm, tag="u",
                                           name=f"u{p}")
                            us.append(u)
                            nc.gpsimd.memset(
                                u[L:L + 1, 0, :, :].bitcast(dt32), 0.0)  # carry0
                        u = us[p]
                        for h in range(2):
                            nc.sync.dma_start(
                                out=u[0:L, c0:c1, h, :],
                                in_=noisep[2 * p + h, :, c0:c1, :])
                # PE warmup during the load ramp: ramps the clock gate so the
                # first real chain matmuls run at full rate
                wps = pspool.tile([L + 1, 2 * D], dt32, tag="ps", name="wps")
                for _ in range(10):
                    nc.tensor.matmul(wps[:, 0:L + 1], w_t[:, 0, :],
                                     w_t[:, 0, :], start=True, stop=True)
                # Skewed chain interleave: pair p runs SKEW chunks behind
                # pair p-1 so early chains aren't head-of-line blocked on
                # later pairs' still-inflight loads.
                SKEW = 3
                for step in range(C + (NP - 1) * SKEW):
                    for p in range(NP):
                        c = step - p * SKEW
                        if not (0 <= c < C):
                            continue
                        u = us[p]
                        ps = pspool.tile([L + 1, 2 * D], dt32,
                                         tag="ps", name=f"ps{p}")
                        nc.tensor.matmul(ps[:], w_t[:, c, :],
                                         u[0:L + 1, c, :, :],
                                         start=True, stop=True)
                        if c + 1 < C:
                            # next chunk's carry = dup'd final-y row
                            nc.scalar.copy(u[L:L + 1, c + 1, :, :],
                                           ps[L:L + 1, :])
                        # evacuate final y back into the dead u slice
                        evac = u[0:L, c, :, :].bitcast(dt32)
                        if c % 2 == 1:
                            nc.scalar.copy(evac, ps[0:L, :])
                        else:
                            nc.vector.tensor_copy(out=evac, in_=ps[0:L, :])
                        if c == CH - 1 or c == C - 1:
                            c0, c1 = (0, CH) if c == CH - 1 else (CH, C)
                            for h in range(2):
                                nc.scalar.dma_start(
                                    out=youtp[2 * p + h, :, c0:c1, :],
                                    in_=u[0:L, c0:c1, h, :].bitcast(dt32))

            if n_iters == 1:
                body()
            else:
                with tc.For_i(0, n_iters, 1) as _i:
                    body(_i)
    nc.compile()
    return nc


def _build_nc(with_ydet, n_iters=1):
    """Bass program for one core. noise/yout are [B_S, TP, D] in DRAM."""
    nc = bacc.Bacc("TRN2", target_bir_lowering=False, debug=False,
                   num_devices=N_CORES)
    dt32 = mybir.dt.float32
    noise = nc.dram_tensor("noise", [B_S, TP, D], dt32, kind="ExternalInput")
    s_u = nc.dram_tensor("s_u", [TP, D], dt32, kind="ExternalInput")
    g = nc.dram_tensor("g", [TP, D], dt32, kind="ExternalInput")
    tri = nc.dram_tensor("tri", [L + 1, L + 1], dt32, kind="ExternalInput")
    ydet = (nc.dram_tensor("ydet", [TP, D], dt32, kind="ExternalInput")
            if with_ydet else None)
    yout = nc.dram_tensor("yout", [B_S, TP, D], dt32, kind="ExternalOutput")

    GRP = 4  # batch rows processed with interleaved carry chains

    with TileContext(nc) as tc:
        with (
            tc.tile_pool(name="coef", bufs=1) as coef,
            tc.tile_pool(name="upool", bufs=GRP + 2) as upool,
            tc.tile_pool(name="psum", bufs=8, space="PSUM") as pspool,
        ):
            def body(_iv=None):
                w_t = coef.tile([L + 1, L + 1], dt32, tag="w")
                su_t = coef.tile([128, C, D], dt32, tag="su")
                g_t = coef.tile([128, C, D], dt32, tag="g")
                nc.sync.dma_start(out=w_t[:], in_=tri[:])
                nc.sync.dma_start(
                    out=su_t[0:L, :, :],
                    in_=s_u[:].rearrange("(c p) d -> p c d", p=L))
                nc.sync.dma_start(
                    out=g_t[0:L, :, :],
                    in_=g[:].rearrange("(c p) d -> p c d", p=L))
                if with_ydet:
                    yd_t = coef.tile([128, C, D], dt32, tag="yd")
                    nc.sync.dma_start(
                        out=yd_t[0:L, :, :],
                        in_=ydet[:].rearrange("(c p) d -> p c d", p=L))

                for g0 in range(0, B_S, GRP):
                    bs = range(g0, min(g0 + GRP, B_S))
                    us = {}
                    for b in bs:
                        u = us[b] = upool.tile([128, C, D], dt32, tag="u", name=f"u{b}")
                        nc.sync.dma_start(
                            out=u[0:L, :, :],
                            in_=noise[b].rearrange("(c p) d -> p c d", p=L))
                        nc.gpsimd.memset(u[L:L + 1, 0, :], 0.0)  # chunk-0 carry
                        nc.vector.tensor_mul(out=u[0:L, :, :], in0=u[0:L, :, :],
                                             in1=su_t[0:L, :, :])
                    # interleave the per-b carry chains chunk-by-chunk; pass3
                    # (y = G*cum, PSUM->SBUF) writes back into the dead u slice
                    pss = {}
                    for c in range(C):
                        h = c % 2
                        for b in bs:
                            u = us[b]
                            if h == 0:
                                pss[b] = pspool.tile([L + 1, 2 * D], dt32, tag="ps", name=f"ps{b}")
                            ps = pss[b]
                            nc.tensor.matmul(ps[:, h * D:(h + 1) * D],
                                             w_t[:], u[0:L + 1, c, :],
                                             start=True, stop=True)
                            if c + 1 < C:
                                # next chunk's additive carry = dup'd cum row
                                nc.scalar.copy(u[L:L + 1, c + 1, :],
                                               ps[L:L + 1, h * D:(h + 1) * D])
                            if h == 1:
                                nc.vector.tensor_mul(
                                    out=u[0:L, c - 1:c + 1, :],
                                    in0=g_t[0:L, c - 1:c + 1, :],
                                    in1=ps[0:L, :])
                    for b in bs:
                        u = us[b]
                        if with_ydet:
                            nc.vector.tensor_add(out=u[0:L, :, :],
                                                 in0=u[0:L, :, :],
                                                 in1=yd_t[0:L, :, :])
                        nc.sync.dma_start(
                            out=yout[b].rearrange("(c p) d -> p c d", p=L),
                            in_=u[0:L, :, :])

            if n_iters == 1:
                body()
            else:
                with tc.For_i(0, n_iters, 1) as _i:
                    body(_i)
    nc.compile()
    return nc


_CACHE = {}


def _get_nc(mode, n_iters=1):
    key = (mode, n_iters)
    if key not in _CACHE:
        if mode == "fastc":
            _CACHE[key] = _build_nc_fastc(n_iters)
        elif mode == "fastb":
            _CACHE[key] = _build_nc_fastb(n_iters)
        elif mode == "fast":
            _CACHE[key] = _build_nc_fast(n_iters)
        elif mode == "fast_f32r":
            _CACHE[key] = _build_nc_fast(n_iters, f32r=True)
        else:
            _CACHE[key] = _build_nc(mode == "general_ydet", n_iters)
    return _CACHE[key]


def _make_in_maps(ts, noise, mu, log_kappa, log_sigma):
    """Returns (in_maps, mode). mode: 'fast' when the per-(t,d) coefficients
    are uniform (uniform time grid, d-uniform kappa/sigma, mu=0) — then the
    exact d-independent chunk weight is used and no coefficient tensors ship."""
    S_u, G, ydet, A_full, sqrtQ_full = _host_coeffs(
        np.asarray(ts), np.asarray(mu),
        np.asarray(log_kappa), np.asarray(log_sigma))
    noise = np.ascontiguousarray(np.asarray(noise), dtype=_f32)

    fast = (ydet is None
            and np.ptp(A_full, axis=1).max() == 0
            and np.ptp(sqrtQ_full, axis=1).max() == 0
            and A_full.min() > 0)
    shards = []
    for core in range(N_CORES):
        shard = noise[core * B_S:(core + 1) * B_S]        # [B_S, T, D]
        npad = np.zeros((B_S, TP, D), _f32)
        npad[:, :T] = shard
        shards.append(npad)

    if fast:
        bf16 = mybir.dt.np(mybir.dt.bfloat16)
        if _FAST_MODE == "fastc":
            ws = _wfold_weights2(A_full, sqrtQ_full)     # [C2, 128, 128]
            ws_t = np.ascontiguousarray(
                ws.transpose(1, 0, 2)).reshape(L2 + 1, C2 * (L2 + 1))
            w1 = np.ascontiguousarray(ws_t[:L2]).astype(bf16)
            w2 = np.ascontiguousarray(ws_t[L2:]).astype(bf16)
            # boundary columns: chunk c's contribution to y at the end of
            # chunk CA-1 = (rolled col 0 of W1_c) x prod of later chunks'
            # full-chunk decays (= rolled W2[r, 0])
            wb = np.stack(
                [ws[c][:L2, 0] *
                 np.prod([ws[r][L2, 0] for r in range(c + 1, CA)])
                 for c in range(CA)], axis=1)            # [L2, CA]
            wb = np.ascontiguousarray(wb).astype(bf16)
            in_maps = []
            NP = B_S // 2
            for s in shards:                             # s: [B_S, TP, D]
                sp = np.zeros((B_S, TP2, D), _f32)
                sp[:, :T] = s[:, :T]
                # [B_S, TP2, D] -> [NP, L2, C2, 2(h), D], split chunk-wise
                full = sp.reshape(NP, 2, C2, L2, D).transpose(0, 3, 2, 1, 4)
                na = np.ascontiguousarray(full[:, :, :CA]).astype(bf16)
                nb = np.ascontiguousarray(full[:, :, CA:]).astype(bf16)
                in_maps.append({"noiseA": na, "noiseB": nb, "wbstack": wb,
                                "w1stack": w1, "w2stack": w2})
            return in_maps, "fastc"
        ws = _wfold_weights(A_full, sqrtQ_full)          # [C, s, t]
        # roll output columns by 1: col 0 = the dup'd chunk-final value (the
        # carry, evac'd to SBUF partition 0 = a legal PE rhs base), cols
        # 1..96 = y[t0..t0+95]
        ws = np.ascontiguousarray(np.roll(ws, 1, axis=2))
        ws_t = np.ascontiguousarray(
            ws.transpose(1, 0, 2)).reshape(L + 1, C * (L + 1))
        w1 = np.ascontiguousarray(ws_t[:L]).astype(bf16)     # noise rows
        w2 = np.ascontiguousarray(ws_t[L:]).astype(bf16)     # carry decay row
        in_maps = []
        NP = B_S // 2
        CH = C // 2
        for s in shards:
            if _DMA_GRAN == "pair":
                # [B_S, TP, D] -> [NP, L, CH*2=C, 2(h), D]
                sp = np.ascontiguousarray(
                    s.reshape(NP, 2, C, L, D).transpose(0, 3, 2, 1, 4)
                ).astype(bf16)
            else:
                # [B_S, TP, D] -> [NP, 2(half), L, CH, 2(h), D]
                sp = np.ascontiguousarray(
                    s.reshape(NP, 2, 2, CH, L, D).transpose(0, 2, 4, 3, 1, 5)
                ).astype(bf16)
            in_maps.append({"noisep": sp, "w1stack": w1, "w2stack": w2})
        return in_maps, "fastb"

    su_p = _pad_tp(S_u)
    g_p = _pad_tp(G)
    yd_p = _pad_tp(ydet) if ydet is not None else None
    tri = _tri_weight()
    in_maps = []
    for s in shards:
        m = {"noise": s, "s_u": su_p, "g": g_p, "tri": tri}
        if yd_p is not None:
            m["ydet"] = yd_p
        in_maps.append(m)
    return in_maps, ("general_ydet" if yd_p is not None else "general")


def kernel(ts, noise, mu, log_kappa, log_sigma):
    in_maps, mode = _make_in_maps(ts, noise, mu, log_kappa, log_sigma)
    nc = _get_nc(mode)
    res = run_bass_kernel_spmd(nc, in_maps, list(range(N_CORES)))
    out = np.empty((B, T, D), _f32)
    for core in range(N_CORES):
        r = res.results[core]
        if mode == "fastc":
            # youtA/B [NP, L2, CA|CB, 2, D] -> [B_S, TP2, D]
            full = np.concatenate(
                [r["youtA"].astype(_f32), r["youtB"].astype(_f32)], axis=2)
            y = full.transpose(0, 3, 2, 1, 4).reshape(B_S, TP2, D)
        elif mode == "fastb":
            if _DMA_GRAN == "pair":
                # [NP, L, C, 2(h), D] -> [B_S, TP, D]
                y = r["youtp"].astype(_f32).transpose(
                    0, 3, 2, 1, 4).reshape(B_S, TP, D)
            else:
                # [NP, 2(half), L, CH, 2(h), D] -> [B_S, TP, D]
                y = r["youtp"].astype(_f32).transpose(
                    0, 4, 1, 3, 2, 5).reshape(B_S, TP, D)
        elif mode == "fast":
            y = r["youtp"].astype(_f32).transpose(0, 2, 1, 3).reshape(
                B_S, TP, D)
        else:
            y = r["yout"]
        out[core * B_S:(core + 1) * B_S] = y[:, :T, :]
    return out

